# revision 10
# baseline (speedup 1.0000x reference)
"""AnchorTarget distributed Bass kernel for 8 TRN2 NeuronCores.

Strategy (per sharding hint): anchors (N=262144) sharded contiguously across
8 cores (32768 each). Each core computes its [32768, 128] IoU rows against
all 128 gt boxes, row-max/argmax, labels and encoded targets locally; the
gt-wise column max is combined with one small AllReduce(max) over [G=128].
The fg/bg random subsampling (which needs a global rank over data-dependent
masks) is finalized on the host from the per-anchor labels.

Device data layout per core: anchor coordinate arrays [128 partitions, 256
blocks]; anchor (block t, partition p) = global index core*32768 + t*128 + p.
Each block computes a [128 anchors, 128 gt] IoU tile with fused DVE ops
(tensor_scalar / scalar_tensor_tensor), exact iterative reciprocal for the
IoU division, row max + first-argmax, a one-hot PE matmul gather of the
argmax gt box, and a valid-masked column-max accumulator. After the
AllReduce, a second pass flags anchors achieving any gt's global max.
"""

import numpy as np

import concourse.bass as bass
import concourse.mybir as mybir
import concourse.tile as tile
from concourse.bacc import Bacc
from concourse.bass_utils import run_bass_kernel_spmd
from concourse.tile import TileContext

NCORES = 8
N = 262144
G = 128
NPC = N // NCORES      # anchors per core
TB = NPC // 128        # blocks per core (each block = 128 anchors)
POS_IOU = 0.7
NEG_IOU = 0.3
TOTAL_SAMPLES = 256
MAX_POS = 128

F32 = mybir.dt.float32
Alu = mybir.AluOpType
Act = mybir.ActivationFunctionType
Ax = mybir.AxisListType

_CACHE = {}


def _build(tb=TB):
    nc = Bacc(None, target_bir_lowering=False, num_devices=NCORES)

    # ---- I/O ----
    a_in = {}
    for k in ("a0", "a1", "a2", "a3"):
        a_in[k] = nc.dram_tensor(k, [128, tb], F32, kind="ExternalInput")
    g_in = {}
    for k in ("g0", "g1", "g2", "g3", "areab", "iotag", "iota1k", "ident"):
        g_in[k] = nc.dram_tensor(k, [128, G], F32, kind="ExternalInput")
    imhw = nc.dram_tensor("imhw", [128, 2], F32, kind="ExternalInput")
    gtkm_in = nc.dram_tensor("gtkm", [128, 4], F32, kind="ExternalInput")

    labels_o = nc.dram_tensor("labels_o", [128, tb], F32, kind="ExternalOutput")
    targets_o = nc.dram_tensor("targets_o", [128, 4 * tb], F32, kind="ExternalOutput")

    with TileContext(nc) as tc:
        with (
            tc.tile_pool(name="const", bufs=1) as constp,
            tc.tile_pool(name="iou", bufs=1) as ioup,
            tc.tile_pool(name="work", bufs=2) as work,
            tc.tile_pool(name="psum", bufs=2, space="PSUM") as psum,
            tc.tile_pool(name="dram", bufs=1, space="DRAM") as dram,
        ):
            # ---- load inputs ----
            A0 = constp.tile([128, tb], F32, tag="A0")
            A1 = constp.tile([128, tb], F32, tag="A1")
            A2 = constp.tile([128, tb], F32, tag="A2")
            A3 = constp.tile([128, tb], F32, tag="A3")
            for t, k in ((A0, "a0"), (A1, "a1"), (A2, "a2"), (A3, "a3")):
                nc.sync.dma_start(t[:], a_in[k][:])
            G0 = constp.tile([128, G], F32, tag="G0")
            G1 = constp.tile([128, G], F32, tag="G1")
            G2 = constp.tile([128, G], F32, tag="G2")
            G3 = constp.tile([128, G], F32, tag="G3")
            AREAB = constp.tile([128, G], F32, tag="AREAB")
            IOTAG = constp.tile([128, G], F32, tag="IOTAG")
            IOTA1K = constp.tile([128, G], F32, tag="IOTA1K")
            IDENT = constp.tile([128, G], F32, tag="IDENT")
            for t, k in ((G0, "g0"), (G1, "g1"), (G2, "g2"), (G3, "g3"),
                         (AREAB, "areab"), (IOTAG, "iotag"), (IOTA1K, "iota1k"),
                         (IDENT, "ident")):
                nc.sync.dma_start(t[:], g_in[k][:])
            HW = constp.tile([128, 2], F32, tag="HW")
            nc.sync.dma_start(HW[:], imhw[:])
            GTKM = constp.tile([128, 4], F32, tag="GTKM")
            nc.sync.dma_start(GTKM[:], gtkm_in[:])

            def ts(out, in0, s1, op0, s2=None, op1=None):
                if op1 is None:
                    nc.vector.tensor_scalar(out, in0, s1, None, op0)
                else:
                    nc.vector.tensor_scalar(out, in0, s1, s2, op0, op1)

            def stt(out, in0, s, in1, op0, op1):
                nc.vector.scalar_tensor_tensor(out, in0, s, in1, op0, op1)

            def tt(out, in0, in1, op):
                nc.vector.scalar_tensor_tensor(out, in0, 0.0, in1, Alu.bypass, op)

            # ---- per-anchor prep ----
            HWB = constp.tile([128, 2], F32, tag="HWB")
            ts(HWB[:], HW[:], -1.0, Alu.add)

            VAL = constp.tile([128, tb], F32, tag="VAL")
            VM1 = constp.tile([128, tb], F32, tag="VM1")
            c0 = work.tile([128, tb], F32, tag="c0", bufs=1)
            c1 = work.tile([128, tb], F32, tag="c1", bufs=1)
            ts(c0[:], A0[:], 0.0, Alu.is_ge)
            ts(c1[:], A1[:], 0.0, Alu.is_ge)
            tt(c0[:], c0[:], c1[:], Alu.mult)
            ts(c1[:], A2[:], HWB[:, 0:1], Alu.is_le)
            tt(c0[:], c0[:], c1[:], Alu.mult)
            ts(c1[:], A3[:], HWB[:, 1:2], Alu.is_le)
            tt(VAL[:], c0[:], c1[:], Alu.mult)
            ts(VM1[:], VAL[:], -1.0, Alu.add)

            AH = constp.tile([128, tb], F32, tag="AH")
            AW = constp.tile([128, tb], F32, tag="AW")
            AREA_A = constp.tile([128, tb], F32, tag="AREA_A")
            AY = constp.tile([128, tb], F32, tag="AY")
            AX = constp.tile([128, tb], F32, tag="AX")
            RAH = constp.tile([128, tb], F32, tag="RAH")
            RAW = constp.tile([128, tb], F32, tag="RAW")
            tt(AH[:], A2[:], A0[:], Alu.subtract)
            ts(AH[:], AH[:], 1.0, Alu.add)
            tt(AW[:], A3[:], A1[:], Alu.subtract)
            ts(AW[:], AW[:], 1.0, Alu.add)
            tt(AREA_A[:], AH[:], AW[:], Alu.mult)
            stt(AY[:], AH[:], 0.5, A0[:], Alu.mult, Alu.add)
            stt(AX[:], AW[:], 0.5, A1[:], Alu.mult, Alu.add)
            nc.vector.reciprocal(RAH[:], AH[:])
            nc.vector.reciprocal(RAW[:], AW[:])

            # ---- main loop: per-block IoU ----
            IOU = ioup.tile([128, 128 * tb], F32, tag="IOU")
            RM = constp.tile([128, tb], F32, tag="RM")
            AMF = constp.tile([128, tb], F32, tag="AMF")
            CM = constp.tile([128, G], F32, tag="CM")
            ts(CM[:], AREAB[:], 0.0, Alu.mult, -2.0, Alu.add)
            tgt_ps = psum.tile([128, 4 * tb], F32, tag="tgt_ps", bufs=1)

            for t in range(tb):
                iou_b = IOU[:, t * G:(t + 1) * G]
                yy1 = work.tile([128, G], F32, tag="yy1")
                hm = work.tile([128, G], F32, tag="hm")
                xx1 = work.tile([128, G], F32, tag="xx1")
                wm = work.tile([128, G], F32, tag="wm")
                ts(yy1[:], G0[:], A0[:, t:t + 1], Alu.max)
                stt(hm[:], G2[:], A2[:, t:t + 1], yy1[:], Alu.min, Alu.subtract)
                ts(xx1[:], G1[:], A1[:, t:t + 1], Alu.max)
                stt(wm[:], G3[:], A3[:, t:t + 1], xx1[:], Alu.min, Alu.subtract)
                h = work.tile([128, G], F32, tag="h")
                w = work.tile([128, G], F32, tag="w")
                nc.scalar.activation(h[:], hm[:], Act.Relu, bias=1.0)
                nc.scalar.activation(w[:], wm[:], Act.Relu, bias=1.0)
                inter = work.tile([128, G], F32, tag="inter")
                union = work.tile([128, G], F32, tag="union")
                tt(inter[:], h[:], w[:], Alu.mult)
                stt(union[:], AREAB[:], AREA_A[:, t:t + 1], inter[:],
                    Alu.add, Alu.subtract)
                rcp = work.tile([128, G], F32, tag="rcp")
                nc.vector.reciprocal(rcp[:], union[:])
                tt(iou_b, inter[:], rcp[:], Alu.mult)
                nc.vector.tensor_reduce(RM[:, t:t + 1], iou_b, axis=Ax.X, op=Alu.max)
                # first-argmax: min over (eq ? g : g+1000)
                cand = work.tile([128, G], F32, tag="cand")
                ts(cand[:], iou_b, RM[:, t:t + 1], Alu.is_equal, -1000.0, Alu.mult)
                tt(cand[:], cand[:], IOTA1K[:], Alu.add)
                nc.vector.tensor_reduce(AMF[:, t:t + 1], cand[:], axis=Ax.X, op=Alu.min)
                # one-hot of first argmax; gather gt row via PE matmul
                oh = work.tile([128, G], F32, tag="oh")
                ts(oh[:], IOTAG[:], AMF[:, t:t + 1], Alu.is_equal)
                ohT_ps = psum.tile([128, G], F32, tag="ohT_ps")
                nc.tensor.transpose(ohT_ps[:], oh[:], IDENT[:])
                ohT = work.tile([128, G], F32, tag="ohT")
                nc.vector.tensor_copy(ohT[:], ohT_ps[:])
                nc.tensor.matmul(tgt_ps[:, t * 4:(t + 1) * 4], ohT[:], GTKM[:],
                                 start=True, stop=True)
                # masked column max accumulate
                ovm = work.tile([128, G], F32, tag="ovm")
                ts(ovm[:], iou_b, VAL[:, t:t + 1], Alu.mult, VM1[:, t:t + 1], Alu.add)
                tt(CM[:], ovm[:], CM[:], Alu.max)

            # copy gathered gt coords out of PSUM (c-fastest layout)
            GAALL = constp.tile([128, 4 * tb], F32, tag="GAALL")
            nc.vector.tensor_copy(GAALL[:], tgt_ps[:])
            ga_r = GAALL[:].rearrange("p (t c) -> p t c", c=4)
            GA = [ga_r[:, :, i] for i in range(4)]

            # ---- cross-partition + cross-core column max ----
            cmt_p = psum.tile([128, 128], F32, tag="cmt", bufs=1)
            nc.tensor.transpose(cmt_p[:], CM[:], IDENT[:])
            CMT = constp.tile([128, 128], F32, tag="CMT")
            nc.vector.tensor_copy(CMT[:], cmt_p[:])
            GTL = constp.tile([128, 1], F32, tag="GTL")
            nc.vector.tensor_reduce(GTL[:], CMT[:], axis=Ax.X, op=Alu.max)

            cc_in = dram.tile([128, 1], F32, tag="cc_in")
            cc_out = dram.tile([128, 1], F32, tag="cc_out")
            nc.sync.dma_start(cc_in[:], GTL[:])
            nc.gpsimd.collective_compute(
                "AllReduce", Alu.max,
                replica_groups=[list(range(NCORES))],
                ins=[cc_in.opt()], outs=[cc_out.opt()])
            GTG = constp.tile([128, 1], F32, tag="GTG")
            nc.sync.dma_start(GTG[:], cc_out[:])

            # replicate [128g,1] -> [128p, 128g]: transpose, then ones-outer-product
            pad = constp.tile([128, 128], F32, tag="pad")
            nc.vector.memset(pad[:], 0.0)
            nc.vector.tensor_copy(pad[:, 0:1], GTG[:])
            padt_p = psum.tile([128, 128], F32, tag="padt", bufs=1)
            nc.tensor.transpose(padt_p[:], pad[:], IDENT[:])
            GTROW = constp.tile([1, 128], F32, tag="GTROW")
            nc.vector.tensor_copy(GTROW[:], padt_p[0:1, :])
            ONESROW = constp.tile([1, 128], F32, tag="ONESROW")
            nc.vector.memset(ONESROW[:], 1.0)
            gtr_ps = psum.tile([128, 128], F32, tag="gtr_ps", bufs=1)
            nc.tensor.matmul(gtr_ps[:], ONESROW[:], GTROW[:], start=True, stop=True)
            GTR = constp.tile([128, 128], F32, tag="GTR")
            nc.vector.tensor_copy(GTR[:], gtr_ps[:])

            # ---- pass 2: gt_best = any_g(iou == gtmax) ----
            GTB = constp.tile([128, tb], F32, tag="GTB")
            for t in range(tb):
                iou_b = IOU[:, t * G:(t + 1) * G]
                sc2 = work.tile([128, G], F32, tag="sc2")
                tt(sc2[:], iou_b, GTR[:], Alu.is_equal)
                nc.vector.tensor_reduce(GTB[:, t:t + 1], sc2[:], axis=Ax.X, op=Alu.max)

            # ---- labels: inner = max(2*pos, isneg) - 1 in {-1,0,1} ----
            isneg = work.tile([128, tb], F32, tag="isneg", bufs=1)
            pos = work.tile([128, tb], F32, tag="pos", bufs=1)
            inner = work.tile([128, tb], F32, tag="inner", bufs=1)
            lab = work.tile([128, tb], F32, tag="lab", bufs=1)
            ts(isneg[:], RM[:], NEG_IOU, Alu.is_lt)
            ts(pos[:], RM[:], POS_IOU, Alu.is_ge)
            tt(pos[:], GTB[:], pos[:], Alu.max)
            stt(inner[:], pos[:], 2.0, isneg[:], Alu.mult, Alu.max)
            stt(lab[:], inner[:], -1.0, VAL[:], Alu.add, Alu.mult)
            tt(lab[:], lab[:], VM1[:], Alu.add)
            nc.sync.dma_start(labels_o[:], lab[:])

            # ---- encode targets (c-major [128, 4, tb]) ----
            TGT = constp.tile([128, 4 * tb], F32, tag="TGT")
            GH = work.tile([128, tb], F32, tag="GH", bufs=1)
            GW = work.tile([128, tb], F32, tag="GW", bufs=1)
            GY = work.tile([128, tb], F32, tag="GY", bufs=1)
            GX = work.tile([128, tb], F32, tag="GX", bufs=1)
            tt(GH[:], GA[2], GA[0], Alu.subtract)
            ts(GH[:], GH[:], 1.0, Alu.add)
            tt(GW[:], GA[3], GA[1], Alu.subtract)
            ts(GW[:], GW[:], 1.0, Alu.add)
            stt(GY[:], GH[:], 0.5, GA[0], Alu.mult, Alu.add)
            stt(GX[:], GW[:], 0.5, GA[1], Alu.mult, Alu.add)
            d0 = work.tile([128, tb], F32, tag="d0", bufs=1)
            q2 = work.tile([128, tb], F32, tag="q2", bufs=1)
            t2 = work.tile([128, tb], F32, tag="t2", bufs=1)
            # ty
            tt(d0[:], GY[:], AY[:], Alu.subtract)
            tt(d0[:], d0[:], RAH[:], Alu.mult)
            tt(TGT[:, 0 * tb:1 * tb], d0[:], VAL[:], Alu.mult)
            # tx
            tt(d0[:], GX[:], AX[:], Alu.subtract)
            tt(d0[:], d0[:], RAW[:], Alu.mult)
            tt(TGT[:, 1 * tb:2 * tb], d0[:], VAL[:], Alu.mult)
            # th
            tt(q2[:], GH[:], RAH[:], Alu.mult)
            nc.scalar.activation(t2[:], q2[:], Act.Ln)
            tt(TGT[:, 2 * tb:3 * tb], t2[:], VAL[:], Alu.mult)
            # tw
            tt(q2[:], GW[:], RAW[:], Alu.mult)
            nc.scalar.activation(t2[:], q2[:], Act.Ln)
            tt(TGT[:, 3 * tb:4 * tb], t2[:], VAL[:], Alu.mult)
            nc.sync.dma_start(targets_o[:], TGT[:])

    nc.finalize()
    return nc


def _prep_in_maps(gt_bboxes, image_shape, all_anchors):
    gt = np.ascontiguousarray(np.asarray(gt_bboxes, np.float32))
    A = np.ascontiguousarray(np.asarray(all_anchors, np.float32))
    hw = np.asarray(image_shape, np.float32).reshape(1, 2)
    one = np.float32(1.0)
    area_b = ((gt[:, 2] - gt[:, 0]) + one) * ((gt[:, 3] - gt[:, 1]) + one)
    rep = lambda v: np.ascontiguousarray(np.broadcast_to(v[None, :], (128, G)),
                                         np.float32)
    gshared = {
        "g0": rep(gt[:, 0]), "g1": rep(gt[:, 1]),
        "g2": rep(gt[:, 2]), "g3": rep(gt[:, 3]),
        "areab": rep(area_b),
        "iotag": rep(np.arange(G, dtype=np.float32)),
        "iota1k": rep(np.arange(G, dtype=np.float32) + 1000.0),
        "ident": np.eye(128, dtype=np.float32),
        "imhw": np.ascontiguousarray(np.broadcast_to(hw, (128, 2))),
        "gtkm": gt.copy(),
    }
    in_maps = []
    for c in range(NCORES):
        sl = A[c * NPC:(c + 1) * NPC].reshape(TB, 128, 4)
        m = dict(gshared)
        for i in range(4):
            m[f"a{i}"] = np.ascontiguousarray(sl[:, :, i].T)
        in_maps.append(m)
    return in_maps


def kernel(gt_bboxes, image_shape, all_anchors, num_anchors=None, _results=None):
    if _results is None:
        if "nc" not in _CACHE:
            _CACHE["nc"] = _build()
        nc = _CACHE["nc"]
        in_maps = _prep_in_maps(gt_bboxes, image_shape, all_anchors)
        res = run_bass_kernel_spmd(nc, in_maps, core_ids=list(range(NCORES)))
        results = res.results
    else:
        results = _results

    labels = np.empty(N, np.float32)
    targets = np.empty((N, 4), np.float32)
    for c in range(NCORES):
        lo = results[c]["labels_o"]            # [128, TB]
        to = results[c]["targets_o"]           # [128, 4*TB]
        labels[c * NPC:(c + 1) * NPC] = lo.T.reshape(NPC)
        targets[c * NPC:(c + 1) * NPC] = (
            to.reshape(128, 4, TB).transpose(2, 0, 1).reshape(NPC, 4))

    # ---- host finalize: random fg/bg subsampling + weights ----
    import jax
    kf, kb = jax.random.split(jax.random.key(42))
    uf = np.asarray(jax.random.uniform(kf, (N,)))
    ub = np.asarray(jax.random.uniform(kb, (N,)))

    lab = labels.astype(np.int32)
    fg = lab == 1
    nfg = int(fg.sum())
    if nfg > MAX_POS:
        thr = np.partition(uf[fg], MAX_POS - 1)[MAX_POS - 1]
        lab[fg & (uf > thr)] = -1
    num_fg = int((lab == 1).sum())
    num_bg = TOTAL_SAMPLES - num_fg
    bg = lab == 0
    nbg = int(bg.sum())
    if nbg > num_bg:
        thr = np.partition(ub[bg], num_bg - 1)[num_bg - 1]
        lab[bg & (ub > thr)] = -1

    labels_out = lab.astype(np.float32)
    num_examples = np.float32((lab >= 0).sum())
    inside_w = np.zeros((N, 4), np.float32)
    inside_w[lab == 1] = 1.0
    outside_w = np.zeros((N, 4), np.float32)
    outside_w[lab >= 0] = np.float32(1.0) / num_examples
    return labels_out, targets, inside_w, outside_w


# revision 11
# speedup vs baseline: 1.4273x; 1.4273x over previous
"""AnchorTarget distributed Bass kernel for 8 TRN2 NeuronCores.

Strategy (per sharding hint): anchors (N=262144) sharded contiguously across
8 cores (32768 each). Each core computes its [32768, 128] IoU rows against
all 128 gt boxes, row-max/argmax, labels and encoded targets locally; the
gt-wise column max is combined with one small AllReduce(max) over [G=128].
The fg/bg random subsampling (which needs a global rank over data-dependent
masks) is finalized on the host from the per-anchor labels.

Device data layout per core: anchor coordinate arrays [128 partitions, 256
blocks]; anchor (block t, partition p) = global index core*32768 + t*128 + p.
Each block computes a [128 anchors, 128 gt] IoU tile with fused DVE ops
(tensor_scalar / scalar_tensor_tensor), exact iterative reciprocal for the
IoU division, row max + first-argmax, a one-hot PE matmul gather of the
argmax gt box, and a valid-masked column-max accumulator. After the
AllReduce, a second pass flags anchors achieving any gt's global max.
"""

import numpy as np

import concourse.bass as bass
import concourse.mybir as mybir
import concourse.tile as tile
from concourse.bacc import Bacc
from concourse.bass_utils import run_bass_kernel_spmd
from concourse.tile import TileContext

NCORES = 8
N = 262144
G = 128
NPC = N // NCORES      # anchors per core
TB = NPC // 128        # blocks per core (each block = 128 anchors)
POS_IOU = 0.7
NEG_IOU = 0.3
TOTAL_SAMPLES = 256
MAX_POS = 128

F32 = mybir.dt.float32
Alu = mybir.AluOpType
Act = mybir.ActivationFunctionType
Ax = mybir.AxisListType

_CACHE = {}


def _build(tb=TB):
    nc = Bacc(None, target_bir_lowering=False, num_devices=NCORES)

    # ---- I/O ----
    a_in = {}
    for k in ("a0", "a1", "a2", "a3"):
        a_in[k] = nc.dram_tensor(k, [128, tb], F32, kind="ExternalInput")
    g_in = {}
    for k in ("g0", "g1", "g2", "g3", "areab", "iotag", "iota1k", "ident"):
        g_in[k] = nc.dram_tensor(k, [128, G], F32, kind="ExternalInput")
    imhw = nc.dram_tensor("imhw", [128, 2], F32, kind="ExternalInput")
    gtkm_in = nc.dram_tensor("gtkm", [128, 4], F32, kind="ExternalInput")

    labels_o = nc.dram_tensor("labels_o", [128, tb], F32, kind="ExternalOutput")
    targets_o = nc.dram_tensor("targets_o", [128, 4 * tb], F32, kind="ExternalOutput")

    with TileContext(nc) as tc:
        with (
            tc.tile_pool(name="const", bufs=1) as constp,
            tc.tile_pool(name="iou", bufs=1) as ioup,
            tc.tile_pool(name="work", bufs=3) as work,
            tc.tile_pool(name="psum", bufs=2, space="PSUM") as psum,
            tc.tile_pool(name="dram", bufs=1, space="DRAM") as dram,
        ):
            # ---- load inputs ----
            A0 = constp.tile([128, tb], F32, tag="A0")
            A1 = constp.tile([128, tb], F32, tag="A1")
            A2 = constp.tile([128, tb], F32, tag="A2")
            A3 = constp.tile([128, tb], F32, tag="A3")
            for t, k in ((A0, "a0"), (A1, "a1"), (A2, "a2"), (A3, "a3")):
                nc.sync.dma_start(t[:], a_in[k][:])
            G0 = constp.tile([128, G], F32, tag="G0")
            G1 = constp.tile([128, G], F32, tag="G1")
            G2 = constp.tile([128, G], F32, tag="G2")
            G3 = constp.tile([128, G], F32, tag="G3")
            AREAB = constp.tile([128, G], F32, tag="AREAB")
            IOTAG = constp.tile([128, G], F32, tag="IOTAG")
            IOTA1K = constp.tile([128, G], F32, tag="IOTA1K")
            IDENT = constp.tile([128, G], F32, tag="IDENT")
            for t, k in ((G0, "g0"), (G1, "g1"), (G2, "g2"), (G3, "g3"),
                         (AREAB, "areab"), (IOTAG, "iotag"), (IOTA1K, "iota1k"),
                         (IDENT, "ident")):
                nc.sync.dma_start(t[:], g_in[k][:])
            HW = constp.tile([128, 2], F32, tag="HW")
            nc.sync.dma_start(HW[:], imhw[:])
            GTKM = constp.tile([128, 4], F32, tag="GTKM")
            nc.sync.dma_start(GTKM[:], gtkm_in[:])

            def ts(out, in0, s1, op0, s2=None, op1=None):
                if op1 is None:
                    nc.vector.tensor_scalar(out, in0, s1, None, op0)
                else:
                    nc.vector.tensor_scalar(out, in0, s1, s2, op0, op1)

            def stt(out, in0, s, in1, op0, op1):
                nc.vector.scalar_tensor_tensor(out, in0, s, in1, op0, op1)

            def tt(out, in0, in1, op):
                nc.vector.scalar_tensor_tensor(out, in0, 0.0, in1, Alu.bypass, op)

            # ---- per-anchor prep ----
            HWB = constp.tile([128, 2], F32, tag="HWB")
            ts(HWB[:], HW[:], -1.0, Alu.add)

            VAL = constp.tile([128, tb], F32, tag="VAL")
            VM1 = constp.tile([128, tb], F32, tag="VM1")
            c0 = work.tile([128, tb], F32, tag="c0", bufs=1)
            c1 = work.tile([128, tb], F32, tag="c1", bufs=1)
            ts(c0[:], A0[:], 0.0, Alu.is_ge)
            ts(c1[:], A1[:], 0.0, Alu.is_ge)
            tt(c0[:], c0[:], c1[:], Alu.mult)
            ts(c1[:], A2[:], HWB[:, 0:1], Alu.is_le)
            tt(c0[:], c0[:], c1[:], Alu.mult)
            ts(c1[:], A3[:], HWB[:, 1:2], Alu.is_le)
            tt(VAL[:], c0[:], c1[:], Alu.mult)
            ts(VM1[:], VAL[:], -1.0, Alu.add)

            AH = constp.tile([128, tb], F32, tag="AH")
            AW = constp.tile([128, tb], F32, tag="AW")
            AREA_A = constp.tile([128, tb], F32, tag="AREA_A")
            AY = constp.tile([128, tb], F32, tag="AY")
            AX = constp.tile([128, tb], F32, tag="AX")
            RAH = constp.tile([128, tb], F32, tag="RAH")
            RAW = constp.tile([128, tb], F32, tag="RAW")
            tt(AH[:], A2[:], A0[:], Alu.subtract)
            ts(AH[:], AH[:], 1.0, Alu.add)
            tt(AW[:], A3[:], A1[:], Alu.subtract)
            ts(AW[:], AW[:], 1.0, Alu.add)
            tt(AREA_A[:], AH[:], AW[:], Alu.mult)
            stt(AY[:], AH[:], 0.5, A0[:], Alu.mult, Alu.add)
            stt(AX[:], AW[:], 0.5, A1[:], Alu.mult, Alu.add)
            nc.vector.reciprocal(RAH[:], AH[:])
            nc.vector.reciprocal(RAW[:], AW[:])

            # ---- main loop: per-block IoU ----
            IOU = ioup.tile([128, 128 * tb], F32, tag="IOU")
            RM = constp.tile([128, tb], F32, tag="RM")
            AMF = constp.tile([128, tb], F32, tag="AMF")
            CM = constp.tile([128, G], F32, tag="CM")
            ts(CM[:], AREAB[:], 0.0, Alu.mult, -2.0, Alu.add)
            tgt_ps = psum.tile([128, 4 * tb], F32, tag="tgt_ps", bufs=1)

            for t in range(tb):
                iou_b = IOU[:, t * G:(t + 1) * G]
                yy1 = work.tile([128, G], F32, tag="yy1")
                hm = work.tile([128, G], F32, tag="hm")
                xx1 = work.tile([128, G], F32, tag="xx1")
                wm = work.tile([128, G], F32, tag="wm")
                ts(yy1[:], G0[:], A0[:, t:t + 1], Alu.max)
                stt(hm[:], G2[:], A2[:, t:t + 1], yy1[:], Alu.min, Alu.subtract)
                ts(xx1[:], G1[:], A1[:, t:t + 1], Alu.max)
                stt(wm[:], G3[:], A3[:, t:t + 1], xx1[:], Alu.min, Alu.subtract)
                h = work.tile([128, G], F32, tag="h")
                w = work.tile([128, G], F32, tag="w")
                nc.scalar.activation(h[:], hm[:], Act.Relu, bias=1.0)
                nc.scalar.activation(w[:], wm[:], Act.Relu, bias=1.0)
                inter = work.tile([128, G], F32, tag="inter")
                union = work.tile([128, G], F32, tag="union")
                tt(inter[:], h[:], w[:], Alu.mult)
                stt(union[:], AREAB[:], AREA_A[:, t:t + 1], inter[:],
                    Alu.add, Alu.subtract)
                rcp = work.tile([128, G], F32, tag="rcp")
                nc.vector.reciprocal(rcp[:], union[:])
                tt(iou_b, inter[:], rcp[:], Alu.mult)
                nc.vector.tensor_reduce(RM[:, t:t + 1], iou_b, axis=Ax.X, op=Alu.max)
                # first-argmax: min over (eq ? g : g+1000)
                cand = work.tile([128, G], F32, tag="cand")
                ts(cand[:], iou_b, RM[:, t:t + 1], Alu.is_equal, -1000.0, Alu.mult)
                tt(cand[:], cand[:], IOTA1K[:], Alu.add)
                nc.vector.tensor_reduce(AMF[:, t:t + 1], cand[:], axis=Ax.X, op=Alu.min)
                # one-hot of first argmax; gather gt row via PE matmul
                oh = work.tile([128, G], F32, tag="oh")
                ts(oh[:], IOTAG[:], AMF[:, t:t + 1], Alu.is_equal)
                ohT_ps = psum.tile([128, G], F32, tag="ohT_ps")
                nc.tensor.transpose(ohT_ps[:], oh[:], IDENT[:])
                ohT = work.tile([128, G], F32, tag="ohT")
                nc.vector.tensor_copy(ohT[:], ohT_ps[:])
                nc.tensor.matmul(tgt_ps[:, t * 4:(t + 1) * 4], ohT[:], GTKM[:],
                                 start=True, stop=True)
                # masked column max accumulate
                ovm = work.tile([128, G], F32, tag="ovm")
                ts(ovm[:], iou_b, VAL[:, t:t + 1], Alu.mult, VM1[:, t:t + 1], Alu.add)
                tt(CM[:], ovm[:], CM[:], Alu.max)

            # copy gathered gt coords out of PSUM (c-fastest layout)
            GAALL = constp.tile([128, 4 * tb], F32, tag="GAALL")
            nc.vector.tensor_copy(GAALL[:], tgt_ps[:])
            ga_r = GAALL[:].rearrange("p (t c) -> p t c", c=4)
            GA = [ga_r[:, :, i] for i in range(4)]

            # ---- cross-partition + cross-core column max ----
            cmt_p = psum.tile([128, 128], F32, tag="cmt", bufs=1)
            nc.tensor.transpose(cmt_p[:], CM[:], IDENT[:])
            CMT = constp.tile([128, 128], F32, tag="CMT")
            nc.vector.tensor_copy(CMT[:], cmt_p[:])
            GTL = constp.tile([128, 1], F32, tag="GTL")
            nc.vector.tensor_reduce(GTL[:], CMT[:], axis=Ax.X, op=Alu.max)

            cc_in = dram.tile([128, 1], F32, tag="cc_in")
            cc_out = dram.tile([128, 1], F32, tag="cc_out")
            nc.sync.dma_start(cc_in[:], GTL[:])
            nc.gpsimd.collective_compute(
                "AllReduce", Alu.max,
                replica_groups=[list(range(NCORES))],
                ins=[cc_in.opt()], outs=[cc_out.opt()])
            GTG = constp.tile([128, 1], F32, tag="GTG")
            nc.sync.dma_start(GTG[:], cc_out[:])

            # replicate [128g,1] -> [128p, 128g]: transpose, then ones-outer-product
            pad = constp.tile([128, 128], F32, tag="pad")
            nc.vector.memset(pad[:], 0.0)
            nc.vector.tensor_copy(pad[:, 0:1], GTG[:])
            padt_p = psum.tile([128, 128], F32, tag="padt", bufs=1)
            nc.tensor.transpose(padt_p[:], pad[:], IDENT[:])
            GTROW = constp.tile([1, 128], F32, tag="GTROW")
            nc.vector.tensor_copy(GTROW[:], padt_p[0:1, :])
            ONESROW = constp.tile([1, 128], F32, tag="ONESROW")
            nc.vector.memset(ONESROW[:], 1.0)
            gtr_ps = psum.tile([128, 128], F32, tag="gtr_ps", bufs=1)
            nc.tensor.matmul(gtr_ps[:], ONESROW[:], GTROW[:], start=True, stop=True)
            GTR = constp.tile([128, 128], F32, tag="GTR")
            nc.vector.tensor_copy(GTR[:], gtr_ps[:])

            # ---- pass 2: gt_best = any_g(iou == gtmax) ----
            GTB = constp.tile([128, tb], F32, tag="GTB")
            for t in range(tb):
                iou_b = IOU[:, t * G:(t + 1) * G]
                sc2 = work.tile([128, G], F32, tag="sc2")
                tt(sc2[:], iou_b, GTR[:], Alu.is_equal)
                nc.vector.tensor_reduce(GTB[:, t:t + 1], sc2[:], axis=Ax.X, op=Alu.max)

            # ---- labels: inner = max(2*pos, isneg) - 1 in {-1,0,1} ----
            isneg = work.tile([128, tb], F32, tag="isneg", bufs=1)
            pos = work.tile([128, tb], F32, tag="pos", bufs=1)
            inner = work.tile([128, tb], F32, tag="inner", bufs=1)
            lab = work.tile([128, tb], F32, tag="lab", bufs=1)
            ts(isneg[:], RM[:], NEG_IOU, Alu.is_lt)
            ts(pos[:], RM[:], POS_IOU, Alu.is_ge)
            tt(pos[:], GTB[:], pos[:], Alu.max)
            stt(inner[:], pos[:], 2.0, isneg[:], Alu.mult, Alu.max)
            stt(lab[:], inner[:], -1.0, VAL[:], Alu.add, Alu.mult)
            tt(lab[:], lab[:], VM1[:], Alu.add)
            nc.sync.dma_start(labels_o[:], lab[:])

            # ---- encode targets (c-major [128, 4, tb]) ----
            TGT = constp.tile([128, 4 * tb], F32, tag="TGT")
            GH = work.tile([128, tb], F32, tag="GH", bufs=1)
            GW = work.tile([128, tb], F32, tag="GW", bufs=1)
            GY = work.tile([128, tb], F32, tag="GY", bufs=1)
            GX = work.tile([128, tb], F32, tag="GX", bufs=1)
            tt(GH[:], GA[2], GA[0], Alu.subtract)
            ts(GH[:], GH[:], 1.0, Alu.add)
            tt(GW[:], GA[3], GA[1], Alu.subtract)
            ts(GW[:], GW[:], 1.0, Alu.add)
            stt(GY[:], GH[:], 0.5, GA[0], Alu.mult, Alu.add)
            stt(GX[:], GW[:], 0.5, GA[1], Alu.mult, Alu.add)
            d0 = work.tile([128, tb], F32, tag="d0", bufs=1)
            q2 = work.tile([128, tb], F32, tag="q2", bufs=1)
            t2 = work.tile([128, tb], F32, tag="t2", bufs=1)
            # ty
            tt(d0[:], GY[:], AY[:], Alu.subtract)
            tt(d0[:], d0[:], RAH[:], Alu.mult)
            tt(TGT[:, 0 * tb:1 * tb], d0[:], VAL[:], Alu.mult)
            # tx
            tt(d0[:], GX[:], AX[:], Alu.subtract)
            tt(d0[:], d0[:], RAW[:], Alu.mult)
            tt(TGT[:, 1 * tb:2 * tb], d0[:], VAL[:], Alu.mult)
            # th
            tt(q2[:], GH[:], RAH[:], Alu.mult)
            nc.scalar.activation(t2[:], q2[:], Act.Ln)
            tt(TGT[:, 2 * tb:3 * tb], t2[:], VAL[:], Alu.mult)
            # tw
            tt(q2[:], GW[:], RAW[:], Alu.mult)
            nc.scalar.activation(t2[:], q2[:], Act.Ln)
            tt(TGT[:, 3 * tb:4 * tb], t2[:], VAL[:], Alu.mult)
            nc.sync.dma_start(targets_o[:], TGT[:])

    nc.finalize()
    return nc


def _prep_in_maps(gt_bboxes, image_shape, all_anchors):
    gt = np.ascontiguousarray(np.asarray(gt_bboxes, np.float32))
    A = np.ascontiguousarray(np.asarray(all_anchors, np.float32))
    hw = np.asarray(image_shape, np.float32).reshape(1, 2)
    one = np.float32(1.0)
    area_b = ((gt[:, 2] - gt[:, 0]) + one) * ((gt[:, 3] - gt[:, 1]) + one)
    rep = lambda v: np.ascontiguousarray(np.broadcast_to(v[None, :], (128, G)),
                                         np.float32)
    gshared = {
        "g0": rep(gt[:, 0]), "g1": rep(gt[:, 1]),
        "g2": rep(gt[:, 2]), "g3": rep(gt[:, 3]),
        "areab": rep(area_b),
        "iotag": rep(np.arange(G, dtype=np.float32)),
        "iota1k": rep(np.arange(G, dtype=np.float32) + 1000.0),
        "ident": np.eye(128, dtype=np.float32),
        "imhw": np.ascontiguousarray(np.broadcast_to(hw, (128, 2))),
        "gtkm": gt.copy(),
    }
    in_maps = []
    for c in range(NCORES):
        sl = A[c * NPC:(c + 1) * NPC].reshape(TB, 128, 4)
        m = dict(gshared)
        for i in range(4):
            m[f"a{i}"] = np.ascontiguousarray(sl[:, :, i].T)
        in_maps.append(m)
    return in_maps


def kernel(gt_bboxes, image_shape, all_anchors, num_anchors=None, _results=None):
    if _results is None:
        if "nc" not in _CACHE:
            _CACHE["nc"] = _build()
        nc = _CACHE["nc"]
        in_maps = _prep_in_maps(gt_bboxes, image_shape, all_anchors)
        res = run_bass_kernel_spmd(nc, in_maps, core_ids=list(range(NCORES)))
        results = res.results
    else:
        results = _results

    labels = np.empty(N, np.float32)
    targets = np.empty((N, 4), np.float32)
    for c in range(NCORES):
        lo = results[c]["labels_o"]            # [128, TB]
        to = results[c]["targets_o"]           # [128, 4*TB]
        labels[c * NPC:(c + 1) * NPC] = lo.T.reshape(NPC)
        targets[c * NPC:(c + 1) * NPC] = (
            to.reshape(128, 4, TB).transpose(2, 0, 1).reshape(NPC, 4))

    # ---- host finalize: random fg/bg subsampling + weights ----
    import jax
    kf, kb = jax.random.split(jax.random.key(42))
    uf = np.asarray(jax.random.uniform(kf, (N,)))
    ub = np.asarray(jax.random.uniform(kb, (N,)))

    lab = labels.astype(np.int32)
    fg = lab == 1
    nfg = int(fg.sum())
    if nfg > MAX_POS:
        thr = np.partition(uf[fg], MAX_POS - 1)[MAX_POS - 1]
        lab[fg & (uf > thr)] = -1
    num_fg = int((lab == 1).sum())
    num_bg = TOTAL_SAMPLES - num_fg
    bg = lab == 0
    nbg = int(bg.sum())
    if nbg > num_bg:
        thr = np.partition(ub[bg], num_bg - 1)[num_bg - 1]
        lab[bg & (ub > thr)] = -1

    labels_out = lab.astype(np.float32)
    num_examples = np.float32((lab >= 0).sum())
    inside_w = np.zeros((N, 4), np.float32)
    inside_w[lab == 1] = 1.0
    outside_w = np.zeros((N, 4), np.float32)
    outside_w[lab >= 0] = np.float32(1.0) / num_examples
    return labels_out, targets, inside_w, outside_w
